# revision 1
# baseline (speedup 1.0000x reference)
"""Causal (cumulative) layer norm kernel for Trainium2, 8 NeuronCores.

Reference semantics (per (b, c) channel, running stats over time t):
    mean_t = cumsum(x)[t] / (t+1)
    var_t  = cumsum(x^2)[t] / (t+1) - mean_t^2
    out    = (x - mean_t) * rsqrt(var_t + 1e-5) * weight + bias

Sharding: data-parallel over batch B=8 -> one batch per core.

Per-core algorithm ([T=4096, C=512], T on SBUF partitions in 32 blocks of 128):
  - x is declared float32r in DRAM (raw fp32 bits feed the fast f32r PE path
    bit-identically to pre-rounded data; measured on HW) and bitcast to fp32
    for the pointwise ops, so no cast ops are needed anywhere.
  - the scan matrices are pre-scaled by 1/n so the PE directly produces
    mean = Uinv@x (+ carry/n) and E[x^2] = Uinv@x^2 (+ carry/n) in PSUM:
    no per-partition-scalar pointwise ops remain, which lets every ACT/DVE
    op run at pair/quad granularity (the per-op fixed overhead measured on
    HW is ~0.3-0.55us, so small ops are poison).
  - two-level scan per wave of 8 blocks: one-hot-window f32r matmuls
    accumulate the 16 block sums of x and x^2 into one PSUM tile; a small
    L-matrix matmul (+ running-total matmul chained across waves) turns
    them into exclusive carries; scaled one-hot selector matmuls broadcast
    carry rows into each block's mean/E[x^2] PSUM accumulation.
  - pointwise: m2 = Square(mean) [ACT pair], var = E[x^2]-m2 [DVE pair],
    xm = x-mean [DVE pair], rstd = Rsqrt(var+eps) [ACT quad],
    out = xm*rstd [DVE/GPS quad].
  - block 0 runs its two scans in true fp32 (4x slower, 2 matmuls): rows
    with t < 128 have catastrophic cancellation in E[x^2]-mean^2 and need
    full precision; rows t >= 128 have concentrated variance so the f32r
    path's ~2^-12 rounding is negligible.
"""
import numpy as np

EPS = 1e-5
B, T, C = 8, 4096, 512
P = 128                 # partitions / block size
NBLK = T // P           # 32
WAVE = 8                # blocks per wave
NWAVE = NBLK // WAVE    # 4

_CACHE = {}


def _build_consts():
    t_idx = np.arange(NBLK * P, dtype=np.float64).reshape(NBLK, P)
    inv_n = (1.0 / (t_idx + 1.0))  # [blk, p]
    U = np.triu(np.ones((P, P), np.float64), k=0)

    # f32r blob [128, RW]: 31 scaled scan matrices | HOT31 | L18 | TOT2 | SEL
    uinv_w = 31 * P
    sel_w = 62 * P
    RW = uinv_w + 32 + 32 + 32 + sel_w
    rb = np.zeros((P, RW), dtype=np.float32)
    for b in range(1, NBLK):
        rb[:, (b - 1) * P:b * P] = (U * inv_n[b][None, :]).astype(np.float32)
    o_hot = uinv_w
    rb[:, o_hot + 15] = 1.0                      # HOT31 at [o_hot : o_hot+31]
    o_l18 = uinv_w + 32
    # L18 lhsT [K=16, M=18]: cols 0..7 = exclusive x-carries, col 8 = x total,
    # cols 9..16 = exclusive q-carries, col 17 = q total
    L18 = np.zeros((16, 18), np.float32)
    for m in range(8):
        L18[0:m, m] = 1.0
        L18[8:8 + m, 9 + m] = 1.0
    L18[0:8, 8] = 1.0
    L18[8:16, 17] = 1.0
    rb[0:16, o_l18:o_l18 + 18] = L18
    o_tot = uinv_w + 64
    rb[8, o_tot:o_tot + 9] = 1.0        # prev totx -> cols 0..8
    rb[17, o_tot + 9:o_tot + 18] = 1.0  # prev totq -> cols 9..17
    o_sel = uinv_w + 96
    # selectors for block b (1..31), within-wave index i = b % 8:
    #   x-sel window 2*(b-1):   row i     = inv_n[b]
    #   q-sel window 2*(b-1)+1: row 9 + i = inv_n[b]
    for b in range(1, NBLK):
        i = b % WAVE
        w0 = o_sel + 2 * (b - 1) * P
        rb[i, w0:w0 + P] = inv_n[b].astype(np.float32)
        rb[9 + i, w0 + P:w0 + 2 * P] = inv_n[b].astype(np.float32)

    # fp32 blob: block 0 raw scan matrix (0/1 weights, matches the
    # reference's cumsum rounding structure) + scalar columns
    fb = np.zeros((P, P + 3), dtype=np.float32)
    fb[:, 0:P] = U.astype(np.float32)
    fb[:, P] = (-inv_n[0]).astype(np.float32)      # -1/n
    fb[:, P + 1] = (t_idx[0] + 1.0).astype(np.float32)   # n
    fb[:, P + 2] = (EPS * (t_idx[0] + 1.0) ** 2).astype(np.float32)  # eps*n^2
    import ml_dtypes
    bb = U.astype(ml_dtypes.bfloat16)
    offs = {"hot": o_hot, "l18": o_l18, "tot": o_tot, "sel": o_sel, "w": RW}
    return fb, rb, bb, offs


def _build_program(iters=1):
    import concourse.bacc as bacc
    import concourse.tile as tile
    from concourse import mybir

    dt = mybir.dt
    AF = mybir.ActivationFunctionType
    ALU = mybir.AluOpType

    _, _, _, offs = _build_consts()
    RW = offs["w"]

    nc = bacc.Bacc()
    x_d = nc.declare_dram_parameter("x", [T, C], dt.float32, isOutput=False)
    fb_d = nc.declare_dram_parameter("fblob", [P, P + 3], dt.float32, isOutput=False)
    bb_d = nc.declare_dram_parameter("bblob", [P, P], dt.bfloat16, isOutput=False)
    rb_d = nc.declare_dram_parameter("rblob", [P, RW], dt.float32r, isOutput=False)
    y_d = nc.declare_dram_parameter("y", [T, C], dt.float32, isOutput=True)

    x_v = x_d[:, :].rearrange("(n p) c -> p n c", p=P)   # [128, 32, 512] f32r
    y_v = y_d[:, :].rearrange("(n p) c -> p n c", p=P)

    def raw_rsqrt(out_ap, in_ap, bias_ap):
        eng = nc.scalar
        ins = [eng.lower_ap(in_ap), eng.lower_ap(bias_ap),
               mybir.ImmediateValue(dtype=dt.float32, value=1.0),
               mybir.ImmediateValue(dtype=dt.float32, value=0.0)]
        return eng.add_instruction(mybir.InstActivation(
            name=nc.get_next_instruction_name(), func=AF.Rsqrt,
            ins=ins, outs=[eng.lower_ap(out_ap)]))

    with tile.TileContext(nc) as tc:
        with (
            tc.tile_pool(name="consts", bufs=1) as consts,
            tc.tile_pool(name="waves", bufs=2) as waves,
            tc.tile_pool(name="blk", bufs=3) as blk,
            tc.tile_pool(name="blk2", bufs=2) as blk2,
            tc.tile_pool(name="blk1", bufs=1) as blk1,
            tc.tile_pool(name="ps_mq", bufs=3, space="PSUM") as ps_mq,
            tc.tile_pool(name="ps_small", bufs=1, space="PSUM") as ps_small,
        ):
            fb = consts.tile([P, P + 3], dt.float32, tag="fb")
            bb = consts.tile([P, P], dt.bfloat16, tag="bb")
            nc.sync.dma_start(out=bb, in_=bb_d[:, :])
            rb = consts.tile([P, RW], dt.float32r, tag="rb")
            nc.sync.dma_start(out=fb, in_=fb_d[:, :])
            nc.sync.dma_start(out=rb, in_=rb_d[:, :])
            eps_t = consts.tile([P, 1], dt.float32, tag="eps")
            nc.vector.memset(eps_t, EPS)

            U0b = bb[:, 0:P]
            neginv0 = fb[:, P:P + 1]
            nvec0 = fb[:, P + 1:P + 2]
            epsn2_0 = fb[:, P + 2:P + 3]

            def Uinv(b):        # [128,128] f32r scaled scan lhsT, b in 1..31
                return rb[:, (b - 1) * P:b * P]

            def HOT(j):         # [128,16] one-hot col j (j in 0..15)
                return rb[:, offs["hot"] + 15 - j:offs["hot"] + 31 - j]

            L18r = rb[0:16, offs["l18"]:offs["l18"] + 18]
            TOT2r = rb[0:18, offs["tot"]:offs["tot"] + 18]

            def SELx(b):        # [18,128] scaled x-carry selector, b in 1..31
                return rb[0:18, offs["sel"] + 2 * (b - 1) * P:
                          offs["sel"] + (2 * (b - 1) + 1) * P]

            def SELq(b):
                return rb[0:18, offs["sel"] + (2 * (b - 1) + 1) * P:
                          offs["sel"] + (2 * b) * P]

            import contextlib
            loop_cm = tc.For_i(0, iters, 1) if iters > 1 else \
                contextlib.nullcontext()
            with loop_cm:
                prev_carr = None
                for w in range(NWAVE):
                    xw = waves.tile([P, WAVE, C], dt.float32, tag="xw")
                    nc.sync.dma_start(
                        out=xw, in_=x_v[:, w * WAVE:(w + 1) * WAVE, :])
                    xw32 = xw
                    # f32r copy for the scan/blocksum matmuls (ACT rounds on
                    # write; the f32r DMA path rounds the data so x must be
                    # loaded fp32 and cast on-chip)
                    xr_q = []
                    for g2 in range(2):
                        t = waves.tile([P, 4, C], dt.float32r, tag="xr")
                        nc.scalar.copy(out=t, in_=xw[:, 4 * g2:4 * g2 + 4, :])
                        xr_q.append(t)
                    # x^2 as f32r, pair granularity (ACT)
                    sq_pairs = []
                    for g in range(4):
                        sq = blk2.tile([P, 2, C], dt.float32r, tag=f"sqp{g}")
                        nc.scalar.square(out=sq, in_=xw32[:, 2 * g:2 * g + 2, :])
                        sq_pairs.append(sq)
                    if w == 0:
                        sq0 = blk1.tile([P, C], dt.float32, tag="sq0")
                        nc.scalar.square(out=sq0, in_=xw32[:, 0, :])

                        def split3(src, pfx):
                            hi = blk1.tile([P, C], dt.bfloat16, tag=pfx + "h")
                            nc.vector.tensor_copy(out=hi, in_=src)
                            r = blk1.tile([P, C], dt.float32, tag="splitr")
                            nc.vector.tensor_tensor(out=r, in0=src, in1=hi,
                                                    op=ALU.subtract)
                            mid = blk1.tile([P, C], dt.bfloat16, tag=pfx + "m")
                            nc.vector.tensor_copy(out=mid, in_=r)
                            lo = blk1.tile([P, C], dt.bfloat16, tag=pfx + "l")
                            nc.vector.tensor_tensor(out=lo, in0=r, in1=mid,
                                                    op=ALU.subtract)
                            return hi, mid, lo

                        x0_3 = split3(xw32[:, 0, :], "x0")
                        sq0_3 = split3(sq0, "q0")

                    # block sums of x and x^2 -> one [16,512] psum tile
                    bs_ps = ps_small.tile([16, C], dt.float32, tag="bs")
                    for i in range(WAVE):
                        nc.tensor.matmul(bs_ps, HOT(i), xr_q[i // 4][:, i % 4, :],
                                         start=(i == 0), stop=False)
                    for i in range(WAVE):
                        nc.tensor.matmul(bs_ps, HOT(8 + i),
                                         sq_pairs[i // 2][:, i % 2, :],
                                         start=False, stop=(i == WAVE - 1))
                    bs_sb = blk.tile([16, C], dt.float32r, tag="bs_sb")
                    nc.vector.tensor_copy(out=bs_sb, in_=bs_ps)

                    # carries [18,512] = L18 @ bs (+ prev totals)
                    ca_ps = ps_small.tile([18, C], dt.float32, tag="carr")
                    first = prev_carr is None
                    nc.tensor.matmul(ca_ps, L18r, bs_sb, start=True, stop=first)
                    if not first:
                        nc.tensor.matmul(ca_ps, TOT2r, prev_carr,
                                         start=False, stop=True)
                    carr = blk.tile([18, C], dt.float32r, tag="carr_sb")
                    nc.vector.tensor_copy(out=carr, in_=ca_ps)
                    prev_carr = carr

                    # pairs: scans + full pointwise at pair granularity
                    for i2 in range(4):
                        mean_ps = ps_mq.tile([P, 2, C], dt.float32, tag="mq")
                        q_ps = ps_mq.tile([P, 2, C], dt.float32, tag="mq")
                        for h in range(2):
                            i = 2 * i2 + h
                            bidx = w * WAVE + i
                            if bidx == 0:
                                # raw S and Q via exact 3-way bf16 splits
                                for j, part in enumerate(x0_3):
                                    nc.tensor.matmul(mean_ps[:, h, :], U0b,
                                                     part, start=(j == 0),
                                                     stop=(j == 2))
                                for j, part in enumerate(sq0_3):
                                    nc.tensor.matmul(q_ps[:, h, :], U0b,
                                                     part, start=(j == 0),
                                                     stop=(j == 2))
                            else:
                                nc.tensor.matmul(mean_ps[:, h, :], Uinv(bidx),
                                                 xr_q[i // 4][:, i % 4, :],
                                                 start=True, stop=False)
                                nc.tensor.matmul(mean_ps[:, h, :], SELx(bidx),
                                                 carr, start=False, stop=True)
                                nc.tensor.matmul(q_ps[:, h, :], Uinv(bidx),
                                                 sq_pairs[i2][:, h, :],
                                                 start=True, stop=False)
                                nc.tensor.matmul(q_ps[:, h, :], SELq(bidx),
                                                 carr, start=False, stop=True)
                        m2 = blk2.tile([P, 2, C], dt.float32, tag="m2")
                        nc.scalar.square(out=m2, in_=mean_ps)
                        var = blk2.tile([P, 2, C], dt.float32, tag="var")
                        nc.vector.tensor_tensor(out=var, in0=q_ps, in1=m2,
                                                op=ALU.subtract)
                        xm = blk2.tile([P, 2, C], dt.float32, tag="xm")
                        nc.vector.tensor_tensor(
                            out=xm, in0=xw32[:, 2 * i2:2 * i2 + 2, :],
                            in1=mean_ps, op=ALU.subtract)
                        if w == 0 and i2 == 0:
                            # half 0 of this pair holds raw S/Q, not
                            # mean/E[x^2]; keep Rsqrt's input in range (the
                            # half-0 result is replaced below)
                            nc.gpsimd.memset(var[:, 0, :], 1.0)
                        rstd = blk2.tile([P, 2, C], dt.float32, tag="rstd")
                        raw_rsqrt(rstd, var, eps_t[:, :])
                        outp = blk2.tile([P, 2, C], dt.float32, tag="outp")
                        eng = nc.vector if i2 % 2 == 0 else nc.gpsimd
                        eng.tensor_tensor(out=outp, in0=xm, in1=rstd,
                                          op=ALU.mult)
                        if w == 0 and i2 == 0:
                            # block 0: v1-style exact path from the raw S/Q
                            # in the pair's half 0; stored separately.
                            s_ps0 = mean_ps[:, 0, :]
                            q_ps0 = q_ps[:, 0, :]
                            xm0 = blk1.tile([P, C], dt.float32, tag="xm0")
                            nc.vector.scalar_tensor_tensor(
                                out=xm0, in0=s_ps0, scalar=neginv0,
                                in1=xw32[:, 0, :], op0=ALU.mult, op1=ALU.add)
                            s2_0 = blk1.tile([P, C], dt.float32, tag="s2_0")
                            nc.scalar.square(out=s2_0, in_=s_ps0)
                            d0 = blk1.tile([P, C], dt.float32, tag="d0")
                            nc.vector.scalar_tensor_tensor(
                                out=d0, in0=q_ps0, scalar=nvec0, in1=s2_0,
                                op0=ALU.mult, op1=ALU.subtract)
                            r0 = blk1.tile([P, C], dt.float32, tag="r0")
                            raw_rsqrt(r0, d0, epsn2_0)
                            out0 = blk1.tile([P, C], dt.float32, tag="out0")
                            nc.vector.scalar_tensor_tensor(
                                out=out0, in0=xm0, scalar=nvec0,
                                in1=r0, op0=ALU.mult, op1=ALU.mult)
                            nc.sync.dma_start(out=y_v[:, 0, :], in_=out0)
                            nc.sync.dma_start(out=y_v[:, 1, :],
                                              in_=outp[:, 1, :])
                        else:
                            nc.sync.dma_start(
                                out=y_v[:, w * WAVE + 2 * i2:
                                        w * WAVE + 2 * i2 + 2, :],
                                in_=outp)
    nc.compile()
    return nc


def kernel(x, weight, bias):
    from concourse.bass_utils import run_bass_kernel_spmd

    x = np.ascontiguousarray(np.asarray(x), dtype=np.float32)
    w = np.asarray(weight, dtype=np.float32).reshape(-1)
    b = np.asarray(bias, dtype=np.float32).reshape(-1)

    if "nc" not in _CACHE:
        fb, rb, bb, _ = _build_consts()
        _CACHE["nc"] = _build_program()
        _CACHE["fb"], _CACHE["rb"], _CACHE["bb"] = fb, rb, bb
    nc = _CACHE["nc"]

    in_maps = [{"x": x[core], "fblob": _CACHE["fb"], "rblob": _CACHE["rb"],
                "bblob": _CACHE["bb"]} for core in range(B)]
    res = run_bass_kernel_spmd(nc, in_maps, list(range(B)))
    y = np.stack([res.results[core]["y"] for core in range(B)], axis=0)

    trivial = np.all(w == 1.0) and np.all(b == 0.0)
    if not trivial:
        y = y * w[None, None, :] + b[None, None, :]
    return y



# revision 7
# speedup vs baseline: 1.4305x; 1.4305x over previous
"""Causal (cumulative) layer norm kernel for Trainium2, 8 NeuronCores.

Reference semantics (per (b, c) channel, running stats over time t):
    mean_t = cumsum(x)[t] / (t+1)
    var_t  = cumsum(x^2)[t] / (t+1) - mean_t^2
    out    = (x - mean_t) * rsqrt(var_t + 1e-5) * weight + bias

Sharding: data-parallel over batch B=8 -> one batch per core.

v2 design (per core, [T=4096, C=512], t-within-block on partitions):
  - f16 I/O: x downcast to f16 on host (plus an exact fp32 copy of the
    first 128 rows for the cancellation-sensitive small-t region); y is
    computed/stored f16 and upcast on host. Halves HBM traffic vs fp32.
  - combined [x | x^2] tiles 1024 wide: every scan/blocksum/selector
    matmul covers both stats in ONE pass (PE cost is out-free-size
    cycles, so this halves instruction count at equal cycles).
  - two-level scan: per wave of 8 blocks, 8 one-hot blocksum matmuls
    accumulate [sum(x)|sum(x^2)] rows 0-7 of a [9,1024] PSUM tile plus a
    ones-column that accumulates the wave total into row 8; a K=1 matmul
    chains the previous wave's total into row 8. One ACT copy moves it
    to SBUF f16. Selector matmuls then inject exclusive carries directly
    from the blocksum rows (carry_i = row8 - sum(rows i..7), scaled by
    1/n per partition) -- no separate carry matmul or extra copy.
  - pointwise per block, f16, spread across engines:
      m2 = Square(mean_ps)      [ACT, PSUM->f16]
      xm = x - mean_ps          [Pool]
      v  = q_ps - m2            [DVE]
      rstd = Rsqrt(v + eps)     [ACT raw activation]
      out  = xm * rstd          [DVE f16 2x]
  - block 0 (t < 128) runs the baseline's exact fp32 path: 3-way bf16
    splits of fp32 x and x^2 feed bit-faithful raw S/Q scans; pointwise
    uses n-scaled scalar_tensor_tensor ops.
"""
import numpy as np

EPS = 1e-5
B, T, C = 8, 4096, 512
P = 128                 # partitions / block size
NBLK = T // P           # 32
WAVE = 8                # blocks per wave
NWAVE = NBLK // WAVE    # 4
W2 = 2 * C              # combined [x | x^2] width

_CACHE = {}


def _build_consts():
    import ml_dtypes
    f16 = ml_dtypes.float16 if not hasattr(np, "float16") else np.float16

    t_idx = np.arange(NBLK * P, dtype=np.float64).reshape(NBLK, P)
    inv_n = 1.0 / (t_idx + 1.0)            # [blk, p]

    # rblob f16 [128, RW]: 31 scan mats | 8 HOT9 | E8 | 31 SEL mats
    o_uinv = 0
    o_hot = o_uinv + 31 * P                # 8 mats of [128, 9]
    o_e8 = o_hot + 8 * 9                   # [1, 9]
    o_sel = o_e8 + 9                       # 31 mats of [9, 128] (rows 0-8)
    RW = o_sel + 31 * P
    rb = np.zeros((P, RW), dtype=np.float16)
    for b in range(1, NBLK):
        w = np.triu(np.ones((P, P), np.float64), k=0) * inv_n[b][None, :]
        rb[:, o_uinv + (b - 1) * P:o_uinv + b * P] = w.astype(np.float16)
    # bs rows: row 0 = running total (chained), rows 1+i = block i sums
    for i in range(WAVE):
        rb[:, o_hot + 9 * i + 1 + i] = 1.0  # one-hot col 1+i (block sum)
        rb[:, o_hot + 9 * i + 0] = 1.0      # ones col 0 (wave total)
    rb[0, o_e8 + 0] = 1.0                   # prev total -> row 0
    for b in range(1, NBLK):
        i = b % WAVE
        s = o_sel + (b - 1) * P
        selw = inv_n[b].astype(np.float16)
        rb[0, s:s + P] = selw               # plus row 0 (total)
        for k in range(i, WAVE):
            rb[1 + k, s:s + P] = -selw      # minus sum(rows 1+i..8)

    # fblob f32 [128, 3]: block-0 scalar columns
    fb = np.zeros((P, 3), dtype=np.float32)
    n0 = t_idx[0] + 1.0
    fb[:, 0] = (-inv_n[0]).astype(np.float32)       # -1/n
    fb[:, 1] = n0.astype(np.float32)                # n
    fb[:, 2] = (EPS * n0 * n0).astype(np.float32)   # eps*n^2

    # bblob bf16 [128, 128]: unscaled upper-tri scan matrix for block 0
    bb = np.triu(np.ones((P, P), np.float64), k=0).astype(ml_dtypes.bfloat16)

    offs = {"uinv": o_uinv, "hot": o_hot, "e8": o_e8, "sel": o_sel, "w": RW}
    return fb, rb, bb, offs


def _build_program(iters=1):
    import concourse.bacc as bacc
    import concourse.tile as tile
    from concourse import mybir

    dt = mybir.dt
    AF = mybir.ActivationFunctionType
    ALU = mybir.AluOpType

    _, _, _, offs = _build_consts()
    RW = offs["w"]

    nc = bacc.Bacc()
    xh_d = nc.declare_dram_parameter("xh", [T, C], dt.float16, isOutput=False)
    x0_d = nc.declare_dram_parameter("x0", [P, C], dt.float32, isOutput=False)
    rb_d = nc.declare_dram_parameter("rblob", [P, RW], dt.float16, isOutput=False)
    fb_d = nc.declare_dram_parameter("fblob", [P, 3], dt.float32, isOutput=False)
    bb_d = nc.declare_dram_parameter("bblob", [P, P], dt.bfloat16, isOutput=False)
    y_d = nc.declare_dram_parameter("y", [T, C], dt.float16, isOutput=True)

    x_v = xh_d[:, :].rearrange("(n p) c -> p n c", p=P)   # [128, 32, 512]
    y_v = y_d[:, :].rearrange("(n p) c -> p n c", p=P)

    def raw_rsqrt(out_ap, in_ap, bias_ap):
        eng = nc.scalar
        ins = [eng.lower_ap(in_ap), eng.lower_ap(bias_ap),
               mybir.ImmediateValue(dtype=dt.float32, value=1.0),
               mybir.ImmediateValue(dtype=dt.float32, value=0.0)]
        return eng.add_instruction(mybir.InstActivation(
            name=nc.get_next_instruction_name(), func=AF.Rsqrt,
            ins=ins, outs=[eng.lower_ap(out_ap)]))

    with tile.TileContext(nc) as tc:
        with (
            tc.tile_pool(name="consts", bufs=1) as consts,
            tc.tile_pool(name="waves", bufs=2) as waves,
            tc.tile_pool(name="bsp", bufs=2) as bsp,
            tc.tile_pool(name="blk", bufs=3) as blk,
            tc.tile_pool(name="blk1", bufs=1) as blk1,
            tc.tile_pool(name="ps_mq", bufs=3, space="PSUM") as ps_mq,
            tc.tile_pool(name="ps_small", bufs=1, space="PSUM") as ps_small,
        ):
            rb = consts.tile([P, RW], dt.float16, tag="rb")
            fb = consts.tile([P, 3], dt.float32, tag="fb")
            bb = consts.tile([P, P], dt.bfloat16, tag="bb")
            nc.sync.dma_start(out=rb, in_=rb_d[:, :])
            nc.sync.dma_start(out=fb, in_=fb_d[:, :])
            nc.sync.dma_start(out=bb, in_=bb_d[:, :])
            eps_t = consts.tile([P, 1], dt.float32, tag="eps")
            nc.vector.memset(eps_t, EPS)

            neginv0 = fb[:, 0:1]
            nvec0 = fb[:, 1:2]
            epsn20 = fb[:, 2:3]

            def Uinv(b):        # [128,128] f16 scaled scan lhsT, b in 1..31
                return rb[:, offs["uinv"] + (b - 1) * P:offs["uinv"] + b * P]

            def HOT(i):         # [128,9] one-hot col i + ones col 8
                return rb[:, offs["hot"] + 9 * i:offs["hot"] + 9 * (i + 1)]

            E8 = rb[0:1, offs["e8"]:offs["e8"] + 9]

            def SEL(b):         # [9,128] carry selector, b in 1..31
                return rb[0:9, offs["sel"] + (b - 1) * P:
                          offs["sel"] + b * P]

            import contextlib
            loop_cm = tc.For_i(0, iters, 1) if iters > 1 else \
                contextlib.nullcontext()
            with loop_cm:
                prev_bs = None
                for w in range(NWAVE):
                    xq = waves.tile([P, WAVE, W2], dt.float16, tag="xq")
                    nc.sync.dma_start(
                        out=xq[:, :, 0:C], in_=x_v[:, w * WAVE:(w + 1) * WAVE, :])
                    # x^2 into the right half (DVE f16 2x)
                    nc.vector.tensor_tensor(
                        out=xq[:, :, C:W2], in0=xq[:, :, 0:C],
                        in1=xq[:, :, 0:C], op=ALU.mult)

                    # blocksums: rows 1-8 = [sum x | sum x^2], row 0 = total
                    # (matmul PSUM output must stay within one 2KB bank, so
                    # every pass is split into 512-wide x and q halves)
                    bs_ps = ps_small.tile([9, W2], dt.float32, tag="small")
                    for h in range(2):
                        hs = bs_ps[:, h * C:(h + 1) * C]
                        for i in range(WAVE):
                            nc.tensor.matmul(hs, HOT(i),
                                             xq[:, i, h * C:(h + 1) * C],
                                             start=(i == 0),
                                             stop=(i == WAVE - 1 and w == 0))
                        if w > 0:
                            nc.tensor.matmul(hs, E8,
                                             prev_bs[0:1, h * C:(h + 1) * C],
                                             start=False, stop=True)
                    bs_sb = bsp.tile([9, W2], dt.float16, tag="bs")
                    nc.scalar.copy(out=bs_sb, in_=bs_ps)
                    prev_bs = bs_sb

                    if w == 0:
                        # block 0 exact fp32 path: inputs
                        x0f = blk1.tile([P, C], dt.float32, tag="x0f")
                        nc.sync.dma_start(out=x0f, in_=x0_d[:, :])
                        sq0 = blk1.tile([P, C], dt.float32, tag="sq0")
                        nc.scalar.square(out=sq0, in_=x0f)

                        def split3(src, pfx):
                            hi = blk1.tile([P, C], dt.bfloat16, tag=pfx + "h")
                            nc.vector.tensor_copy(out=hi, in_=src)
                            r = blk1.tile([P, C], dt.float32, tag=pfx + "r")
                            nc.vector.tensor_tensor(out=r, in0=src, in1=hi,
                                                    op=ALU.subtract)
                            mid = blk1.tile([P, C], dt.bfloat16, tag=pfx + "m")
                            nc.vector.tensor_copy(out=mid, in_=r)
                            lo = blk1.tile([P, C], dt.bfloat16, tag=pfx + "l")
                            nc.vector.tensor_tensor(out=lo, in0=r, in1=mid,
                                                    op=ALU.subtract)
                            return hi, mid, lo

                        x0_3 = split3(x0f, "x0")
                        sq0_3 = split3(sq0, "q0")

                    for i in range(WAVE):
                        b = w * WAVE + i
                        mq = ps_mq.tile([P, W2], dt.float32, tag="mq")
                        if b == 0:
                            # raw S | Q via exact 3-way bf16 splits
                            for j, part in enumerate(x0_3):
                                nc.tensor.matmul(mq[:, 0:C], bb, part,
                                                 start=(j == 0), stop=(j == 2))
                            for j, part in enumerate(sq0_3):
                                nc.tensor.matmul(mq[:, C:W2], bb, part,
                                                 start=(j == 0), stop=(j == 2))
                            s_ps = mq[:, 0:C]
                            q_ps = mq[:, C:W2]
                            xm0 = blk1.tile([P, C], dt.float32, tag="xm0")
                            nc.vector.scalar_tensor_tensor(
                                out=xm0, in0=s_ps, scalar=neginv0, in1=x0f,
                                op0=ALU.mult, op1=ALU.add)
                            s2_0 = blk1.tile([P, C], dt.float32, tag="s2_0")
                            nc.scalar.square(out=s2_0, in_=s_ps)
                            d0 = blk1.tile([P, C], dt.float32, tag="d0")
                            nc.vector.scalar_tensor_tensor(
                                out=d0, in0=q_ps, scalar=nvec0, in1=s2_0,
                                op0=ALU.mult, op1=ALU.subtract)
                            r0 = blk1.tile([P, C], dt.float32, tag="r0")
                            raw_rsqrt(r0, d0, epsn20)
                            out0 = blk1.tile([P, C], dt.float16, tag="out0")
                            nc.vector.scalar_tensor_tensor(
                                out=out0, in0=xm0, scalar=nvec0, in1=r0,
                                op0=ALU.mult, op1=ALU.mult)
                            nc.sync.dma_start(out=y_v[:, 0, :], in_=out0)
                            continue
                        for h in range(2):
                            hs = mq[:, h * C:(h + 1) * C]
                            nc.tensor.matmul(hs, Uinv(b),
                                             xq[:, i, h * C:(h + 1) * C],
                                             start=True, stop=False)
                            nc.tensor.matmul(hs, SEL(b),
                                             bs_sb[:, h * C:(h + 1) * C],
                                             start=False, stop=True)
                        mean_ps = mq[:, 0:C]
                        q_ps = mq[:, C:W2]
                        m2 = blk.tile([P, C], dt.float16, tag="m2")
                        nc.scalar.square(out=m2, in_=mean_ps)
                        xm = blk.tile([P, C], dt.float16, tag="xm")
                        nc.vector.tensor_tensor(out=xm, in0=xq[:, i, 0:C],
                                                in1=mean_ps, op=ALU.subtract)
                        v = blk.tile([P, C], dt.float16, tag="v")
                        nc.vector.tensor_tensor(out=v, in0=q_ps, in1=m2,
                                                op=ALU.subtract)
                        rstd = blk.tile([P, C], dt.float16, tag="rstd")
                        raw_rsqrt(rstd, v, eps_t[:, :])
                        outp = blk.tile([P, C], dt.float16, tag="outp")
                        nc.gpsimd.tensor_tensor(out=outp, in0=xm, in1=rstd,
                                                op=ALU.mult)
                        nc.sync.dma_start(out=y_v[:, b, :], in_=outp)
    nc.compile()
    return nc


def kernel(x, weight, bias):
    from concourse.bass_utils import run_bass_kernel_spmd

    x = np.asarray(x, dtype=np.float32)
    w = np.asarray(weight, dtype=np.float32).reshape(-1)
    b = np.asarray(bias, dtype=np.float32).reshape(-1)

    if "nc" not in _CACHE:
        fb, rb, bb, _ = _build_consts()
        _CACHE["nc"] = _build_program()
        _CACHE["consts"] = {"rblob": rb, "fblob": fb, "bblob": bb}
    nc = _CACHE["nc"]

    xh = x.astype(np.float16)
    in_maps = [{"xh": np.ascontiguousarray(xh[core]),
                "x0": np.ascontiguousarray(x[core, 0:P]),
                **_CACHE["consts"]} for core in range(B)]
    res = run_bass_kernel_spmd(nc, in_maps, list(range(B)))
    y = np.stack([res.results[core]["y"] for core in range(B)], axis=0)
    y = y.astype(np.float32)

    trivial = np.all(w == 1.0) and np.all(b == 0.0)
    if not trivial:
        y = y * w[None, None, :] + b[None, None, :]
    return y
